# revision 4
# baseline (speedup 1.0000x reference)
"""Segment-max pooling (wordpiece->word) Bass kernel for TRN2, 8 cores.

Strategy: pure data parallel, 2 examples per core, fp16 on device.

Host planning (per core, both examples pooled together):
  - nonempty spans of length >= 2 become "lanes" (spans longer than
    RMAX=8 are split into <=RMAX chunks chained through extra rows and
    max-combined on the host; singleton spans/chunk remainders are
    copied from fp32 context on the host -- no device traffic),
  - lanes are sorted by length (desc) and packed into groups of 128
    (one lane per SBUF partition),
  - per group, a quantum Q <= min(lane length) is chosen and each
    indirect-DMA descriptor gathers Q CONSECUTIVE context rows
    (rows s+min(r*Q, len-Q) .. +Q stay inside the span; overlap
    re-reads are harmless for max). ceil(maxlen/Q) gather rounds cover
    the group, so a group of uniform length-8 chain lanes costs ONE
    SWDGE instruction instead of eight. Q trades DMA bytes
    (Q*ceil(L/Q) rows/lane) against Pool-engine SWDGE time
    (~1us/instruction); the planner enumerates Q per group.
  - the gather ucode only accepts single-column offset APs (one index
    per partition), hence one instruction per (group, round).

Device per group:
  - rounds[g] indirect gathers -> [128, rounds*Q, D] scratch slabs,
  - a DVE tensor_max fold tree halves the slab count per op (fp16 gets
    the 2x_1p DVE mode); the last fold writes the group's result slab,
  - a per-group store ships the result, overlapping later gathers.

fp16 context halves the DMA traffic vs fp32. max() over fp16-rounded
values is exactly fp16(true max) (rounding is monotone), and the host
patches the few elements with |v| < 1e-5 from fp32 context, so
per-element relative error stays <= ~3e-3 even in the subnormal range.

Sync-wait budget: the walrus codegen used by the bass2jax/axon path
allows a single attached sync wait per instruction; _split_waits hoists
any extra Tile-generated waits into standalone EventSemaphore
instructions on the same engine queue.
"""

import sys

if "/opt/trn_rl_repo" not in sys.path:
    sys.path.insert(0, "/opt/trn_rl_repo")

import numpy as np

B, S, D, N = 16, 4096, 1024, 1024
NCORES = 8
EPC = B // NCORES  # examples per core
RMAX = 8
PAD_GIDX = 100000  # > EPC*S-1, within int32 after *D
TINY = 1e-5  # host-patch threshold for fp16 subnormal outputs
POOL_W = 0.4  # SWDGE-instruction weight in the per-group quantum choice

_CACHE = {}
LAST_RESULTS = None


def _pick_q(L, m):
    """Quantum for a group with max lane length L, min lane length m."""
    best, best_cost = 1, None
    for q in range(1, m + 1):
        rnds = -(-L // q)
        cost = 728.0 * q * rnds + POOL_W * 1040.0 * rnds
        if best_cost is None or cost < best_cost or (cost == best_cost and q > best):
            best, best_cost = q, cost
    return best


def _plan(spans):
    spans = np.asarray(spans).astype(np.int64)
    # ---- per-example span triage ----------------------------------------
    fixups = [[] for _ in range(B)]  # (span_i, chain_rows, host_tokens)
    nchain = [0] * B
    lanes = [[] for _ in range(NCORES)]  # (length, ex, start, b, row)
    for b in range(B):
        c, e = divmod(b, EPC)
        st = spans[b, :, 0]
        ln = spans[b, :, 1] - st
        for i in np.nonzero(ln > 0)[0]:
            s = int(st[i])
            l = int(ln[i])
            if l == 1:
                fixups[b].append((int(i), [], [s]))
            elif l <= RMAX:
                lanes[c].append((l, e, s, b, int(i)))
            else:
                rows = []
                toks = []
                for o in range(0, l, RMAX):
                    ls = min(RMAX, l - o)
                    if ls == 1:
                        toks.append(s + o)
                        continue
                    row = N + nchain[b]
                    nchain[b] += 1
                    lanes[c].append((ls, e, s + o, b, row))
                    rows.append(row)
                fixups[b].append((int(i), rows, toks))
    for c in range(NCORES):
        lanes[c].sort(key=lambda t: -t[0])

    # ---- static cross-core group structure ------------------------------
    G0 = max(-(-len(lanes[c]) // 128) for c in range(NCORES))
    groups = []  # (Q, rounds, slabs, scroff)
    scroff = 0
    for g in range(G0):
        L = 0
        m = RMAX
        for c in range(NCORES):
            grp = lanes[c][g * 128 : (g + 1) * 128]
            if grp:
                L = max(L, grp[0][0])
                m = min(m, grp[-1][0])
        Q = _pick_q(L, m)
        rnds = -(-L // Q)
        groups.append((Q, rnds, rnds * Q, scroff))
        scroff += rnds * Q
    TOTSCR = scroff
    tot_instr = sum(gr[1] for gr in groups)

    # ---- per-core gather indices + host lane map ------------------------
    gidx = np.full((NCORES, 128, max(tot_instr, 1)), PAD_GIDX, np.int32)
    lane_b = np.full((NCORES, G0 * 128), -1, np.int64)
    lane_row = np.full((NCORES, G0 * 128), -1, np.int64)
    for c in range(NCORES):
        col = 0
        for g, (Q, rnds, slabs, off) in enumerate(groups):
            grp = lanes[c][g * 128 : (g + 1) * 128]
            for p, (l, e, s, b, row) in enumerate(grp):
                j = g * 128 + p
                lane_b[c, j] = b
                lane_row[c, j] = row
                for r in range(rnds):
                    gidx[c, p, col + r] = e * S + s + min(r * Q, l - Q)
            col += rnds

    sig = tuple(groups)
    return sig, G0, groups, TOTSCR, tot_instr, gidx, lane_b, lane_row, fixups, nchain


def _split_waits(nc):
    """Give every instruction at most one attached sync wait.

    The walrus codegen used by the bass2jax/axon path accepts a single
    sync-wait command per instruction, but Tile's add_semaphores may
    attach several (multiple DMA completion lanes, cross-engine deps).
    Semantics-preserving fix: keep one wait attached and hoist the rest
    into standalone InstEventSemaphore instructions inserted directly
    before the instruction on the same engine queue -- the sequencer
    executes them in order, so the wait set is unchanged.
    """
    from concourse import mybir

    used = set()
    for bb in nc.main_func.blocks:
        for ins in bb.instructions:
            si = ins.sync_info
            if si is not None:
                for w in si.on_wait:
                    used.add(w.id)
                for u in si.on_update:
                    used.add(u.id)
    ws_id = max(used) + 1 if used else 0
    for bb in nc.main_func.blocks:
        insts = bb.instructions
        targets = []
        for pos, ins in enumerate(insts):
            si = ins.sync_info
            if si is not None and len(si.on_wait) > 1:
                targets.append((pos, ins))
        for pos, ins in reversed(targets):
            si = ins.sync_info
            waits = list(si.on_wait)
            keep = waits[-1]
            extra = waits[:-1]
            while len(si.on_wait) > 0:
                si.on_wait.pop()
            si.on_wait.append(keep)
            SyncInfo = type(si)
            SyncUpdate = type(si.on_update[0]) if si.on_update else None
            for k, w in enumerate(extra):
                ev = mybir.InstEventSemaphore(name=f"WS{k}-{ins.name}")
                ev.engine = ins.engine
                upd = (
                    [
                        SyncUpdate(
                            sync_type="semaphore",
                            id=ws_id,
                            ant_name="ws_split",
                            update_mode="sem-inc",
                            update_value=1,
                        )
                    ]
                    if SyncUpdate is not None
                    else []
                )
                ev.sync_info = SyncInfo(on_wait=[w], on_update=upd)
                insts.insert(pos, ev)
                nc.inst_map[ev.name] = ev
    return nc


def _build(G0, groups, TOTSCR, tot_instr):
    from concourse import bass, mybir, tile

    nc = bass.Bass()
    f16 = mybir.dt.float16
    i32 = mybir.dt.int32
    ctx_t = nc.declare_dram_parameter("ctx", [EPC * S, D], f16, isOutput=False)
    gidx_t = nc.declare_dram_parameter("gidx", [128, tot_instr], i32, isOutput=False)
    out_t = [
        nc.declare_dram_parameter(f"out{g}", [128, D], f16, isOutput=True)
        for g in range(G0)
    ]
    with tile.TileContext(nc) as tc:
        with tc.tile_pool(name="sbuf", bufs=1) as pool:
            nc.gpsimd.preamble()  # register init for bounds_check scalars
            breg = nc.gpsimd.to_reg(EPC * S - 1)
            # 2D tiles only: the gather ucode requires a flat [128, bytes]
            # out AP (one descriptor per partition, consecutive rows)
            gt = pool.tile([128, tot_instr], i32, tag="gidx")
            res = pool.tile([128, G0 * D], f16, tag="res")
            scr = pool.tile([128, TOTSCR * D], f16, tag="scr")
            nc.sync.dma_start(out=gt[:, :], in_=gidx_t[:, :])
            col = 0
            for g, (Q, rnds, slabs, off) in enumerate(groups):
                for r in range(rnds):
                    a = (off + r * Q) * D
                    nc.gpsimd.indirect_dma_start(
                        out=scr[:, a : a + Q * D],
                        out_offset=None,
                        in_=ctx_t[:],
                        in_offset=bass.IndirectOffsetOnAxis(
                            ap=gt[:, col + r : col + r + 1], axis=0
                        ),
                        bounds_check=breg,
                        oob_is_err=False,
                    )
                col += rnds
                # fold tree: halve the live slab count per op; the final
                # fold writes the group's result slab
                n = slabs
                while n > 1:
                    h = n // 2
                    if n == 2:
                        nc.vector.tensor_max(
                            out=res[:, g * D : (g + 1) * D],
                            in0=scr[:, off * D : (off + 1) * D],
                            in1=scr[:, (off + 1) * D : (off + 2) * D],
                        )
                    else:
                        nc.vector.tensor_max(
                            out=scr[:, off * D : (off + h) * D],
                            in0=scr[:, off * D : (off + h) * D],
                            in1=scr[:, (off + n - h) * D : (off + n) * D],
                        )
                    n -= h
                nc.sync.dma_start(
                    out=out_t[g][:, :], in_=res[:, g * D : (g + 1) * D]
                )
    return _split_waits(nc)


def kernel(context, spans, trace=False):
    global LAST_RESULTS
    context = np.ascontiguousarray(np.asarray(context, dtype=np.float32))
    ctx16 = context.astype(np.float16)
    spans_np = np.asarray(spans)
    (
        sig,
        G0,
        groups,
        TOTSCR,
        tot_instr,
        gidx,
        lane_b,
        lane_row,
        fixups,
        nchain,
    ) = _plan(spans_np)

    out = np.zeros((B, S, D), np.float32)
    maxchain = max(nchain) if max(nchain) else 0
    pooled = np.zeros((B, N + maxchain, D), np.float32)

    if G0 > 0:
        if sig not in _CACHE:
            _CACHE[sig] = _build(G0, groups, TOTSCR, tot_instr)
        nc = _CACHE[sig]

        from concourse.bass_utils import run_bass_kernel_spmd

        in_maps = [
            {
                "ctx": ctx16[c * EPC : (c + 1) * EPC].reshape(EPC * S, D),
                "gidx": gidx[c],
            }
            for c in range(NCORES)
        ]
        LAST_RESULTS = run_bass_kernel_spmd(
            nc, in_maps, list(range(NCORES)), trace=trace
        )
        res = LAST_RESULTS.results

        for c in range(NCORES):
            resv = np.stack(
                [res[c][f"out{g}"] for g in range(G0)], axis=1
            )  # [128, G0, D]
            flat = resv.transpose(1, 0, 2).reshape(G0 * 128, D)
            valid = lane_b[c] >= 0
            pooled[lane_b[c][valid], lane_row[c][valid]] = flat[valid].astype(
                np.float32
            )

    for b in range(B):
        out[b, :N] = pooled[b, :N]
        for i, rows, toks in fixups[b]:
            cands = []
            if rows:
                cands.append(pooled[b, rows].max(axis=0))
            if toks:
                cands.append(context[b, toks].max(axis=0))
            out[b, i] = cands[0] if len(cands) == 1 else np.maximum(cands[0], cands[1])

    # fp16 subnormal patch: for device-pooled spans, recompute elements whose
    # magnitude is below TINY from the fp32 context (handful of elements).
    st = spans_np[..., 0].astype(np.int64)
    en = spans_np[..., 1].astype(np.int64)
    dev_span = (en - st) >= 2  # [B, N]
    cand = np.argwhere(dev_span[:, :, None] & (np.abs(out[:, :N]) < TINY))
    if len(cand):
        by_span = {}
        for b, i, d in cand:
            by_span.setdefault((b, i), []).append(d)
        for (b, i), ds in by_span.items():
            out[b, i, ds] = context[b, st[b, i] : en[b, i], ds].max(axis=0)
    return out
